# revision 8
# baseline (speedup 1.0000x reference)
"""GPT-2 style causal self-attention block on 8 Trainium2 NeuronCores.

Problem: x[4,2048,768] -> qkv = x@c_attn_w + b -> 12-head causal attention
-> a@c_proj_w + b.  Sharding: batch (4) x head-group (2x6 heads) = 8 cores.
Each core computes its batch's qkv columns for its 6 heads, runs attention
for those heads over the full sequence, and produces a partial c_proj
output (contraction over its 384 of 768 a-dims).  The two partials per
batch are summed on the host (+ c_proj bias).

Per-core layouts (all matmul operands bf16, f32 PSUM accumulation):
  qT,kT  [384, 2048]  head-dim on partitions (3 chunks of 128 = head pairs)
  V_aug  [2048, 390]  rows on partitions, per head 64 v-cols + ones col
                      (ones col + biases baked in via an appended ones row
                      of xT and a bias row in the weight)
  S^T    [128k, 512q] psum blocks; exp on ACT (scale=1/8 folded in);
                      causal diag masked via DVE add of a [128,128] tri mask
  A.V    psum [65, 512] accumulated over k-blocks; row 64 = softmax denom
  out^T  bf16 = num * reciprocal(denom) broadcast; feeds c_proj as lhsT
"""

import numpy as np
import ml_dtypes

B, S, D = 4, 2048, 768
NH, DH = 12, 64
NCORES = 8
HPC = 6          # heads per core
PAIRS = 3        # head pairs per core
NQ = S // 512    # q superblocks
NKB = S // 128   # k blocks
BF16 = ml_dtypes.bfloat16

_COMPILED = {}


def _build_program():
    import concourse.mybir as mybir
    import concourse.tile as tile
    from concourse import bacc

    F32, B16 = mybir.dt.float32, mybir.dt.bfloat16
    EXP = mybir.ActivationFunctionType.Exp
    ADD, MULT = mybir.AluOpType.add, mybir.AluOpType.mult

    nc = bacc.Bacc(None, target_bir_lowering=False, debug=False)
    xt_d = nc.dram_tensor("xt", [D, S], B16, kind="ExternalInput")
    wqk_d = nc.dram_tensor("wqk", [D, 768], B16, kind="ExternalInput")
    wqkb_d = nc.dram_tensor("wqkb", [128, 6], F32, kind="ExternalInput")
    wva_d = nc.dram_tensor("wva", [D + 1, HPC * 65], B16, kind="ExternalInput")
    wp_d = nc.dram_tensor("wp", [PAIRS * 128, D], B16, kind="ExternalInput")
    mask_d = nc.dram_tensor("mask", [128, 128], F32, kind="ExternalInput")
    out_d = nc.dram_tensor("out", [S, D], F32, kind="ExternalOutput")

    KC = D // 128  # 6 contraction chunks

    with tile.TileContext(nc) as tc:
        with (
            tc.tile_pool(name="const", bufs=1) as cst,
            tc.tile_pool(name="acts", bufs=1) as acts,
            tc.tile_pool(name="pt", bufs=4) as ptp,
            tc.tile_pool(name="nrm", bufs=3) as nrm,
            tc.tile_pool(name="mm_ps", bufs=2, space="PSUM") as mm_ps,
            tc.tile_pool(name="s_ps", bufs=2, space="PSUM") as s_ps,
            tc.tile_pool(name="o_ps", bufs=2, space="PSUM") as o_ps,
        ):
            xt = cst.tile([128, KC, S], B16, tag="xt", name="xt")
            ones = cst.tile([1, S], B16, tag="ones", name="ones")
            wqk = cst.tile([128, KC, 768], B16, tag="wqk", name="wqk")
            wqkb = cst.tile([128, 6], F32, tag="wqkb", name="wqkb")
            wva = cst.tile([128, KC, HPC * 65], B16, tag="wva", name="wva")
            wvab = cst.tile([1, HPC * 65], B16, tag="wvab", name="wvab")
            wp = cst.tile([128, PAIRS, D], B16, tag="wp", name="wp")
            mask = cst.tile([128, 128], F32, tag="mask", name="mask")

            nc.sync.dma_start(xt[:], xt_d.rearrange("(c p) n -> p c n", p=128))
            nc.sync.dma_start(wqk[:], wqk_d.rearrange("(c p) n -> p c n", p=128))
            nc.sync.dma_start(wqkb[:], wqkb_d[:])
            nc.sync.dma_start(wva[:], wva_d[0:D].rearrange("(c p) n -> p c n", p=128))
            nc.sync.dma_start(wvab[:], wva_d[D:D + 1])
            nc.sync.dma_start(wp[:], wp_d.rearrange("(c p) n -> p c n", p=128))
            nc.sync.dma_start(mask[:], mask_d[:])
            nc.vector.memset(ones[:], 1.0)

            qT = [acts.tile([128, S], B16, tag=f"qT{p}", name=f"qT{p}") for p in range(PAIRS)]
            kT = [acts.tile([128, S], B16, tag=f"kT{p}", name=f"kT{p}") for p in range(PAIRS)]
            vt = [acts.tile([128, HPC * 65], B16, tag=f"v{r}", name=f"v{r}") for r in range(NKB)]
            aT = [acts.tile([128, S], B16, tag=f"aT{p}", name=f"aT{p}") for p in range(PAIRS)]

            def emit_qk_pair(p):
                for dst, m in ((qT[p], p), (kT[p], PAIRS + p)):
                    for n in range(NQ):
                        ps = mm_ps.tile([128, 512], F32, tag="mm", name="mm")
                        for c in range(KC):
                            nc.tensor.matmul(
                                ps[:],
                                wqk[:, c, 128 * m:128 * m + 128],
                                xt[:, c, 512 * n:512 * n + 512],
                                start=(c == 0), stop=(c == KC - 1),
                            )
                        nc.vector.tensor_scalar_add(
                            dst[:, 512 * n:512 * n + 512], ps[:], wqkb[:, m:m + 1])

            def emit_v():
                for r in range(NKB):
                    ps = mm_ps.tile([128, 512], F32, tag="mm", name="mm")
                    pv = ps[:, 0:HPC * 65]
                    for c in range(KC):
                        nc.tensor.matmul(
                            pv, xt[:, c, 128 * r:128 * r + 128], wva[:, c, :],
                            start=(c == 0), stop=False)
                    nc.tensor.matmul(
                        pv, ones[:, 128 * r:128 * r + 128], wvab[:],
                        start=False, stop=True)
                    nc.vector.tensor_copy(vt[r][:], pv)

            def emit_head_J(p, hh, J):
                """One (head, q-superblock): S^T blocks, exp, A.V, normalize."""
                h = 2 * p + hh
                pb = 64 * hh  # partition base of this head in its pair chunk
                nkb = 4 * J + 4
                q_sl = slice(512 * J, 512 * J + 512)
                ps_o = o_ps.tile([128, 512], F32, tag="o", name="o")

                groups = [list(range(g, min(g + 2, nkb))) for g in range(0, nkb, 2)]
                stage = []  # (kbs, ps_s, window_start)

                def s_group(kbs):
                    ps_s = s_ps.tile([128, 1024], F32, tag="s", name="s")
                    w0 = 1024
                    for i, kb in enumerate(kbs):
                        sl = slice(512 * i, 512 * i + 512)
                        nc.tensor.matmul(
                            ps_s[:, sl],
                            kT[p][pb:pb + 64, 128 * kb:128 * kb + 128],
                            qT[p][pb:pb + 64, q_sl],
                            start=True, stop=True,
                            tile_position=(pb, 0),
                        )
                        o = kb - 4 * J
                        if o >= 0:  # diagonal block: add triangular mask
                            nc.vector.tensor_tensor(
                                out=ps_s[:, 512 * i + 128 * o:512 * i + 128 * o + 128],
                                in0=ps_s[:, 512 * i + 128 * o:512 * i + 128 * o + 128],
                                in1=mask[:], op=ADD)
                        # window start: skip 512-col slices that are fully masked
                        w0 = min(w0, 512 * i + (128 * o if o > 0 else 0))
                    return ps_s, w0

                def av_group(kbs, ps_s, w0):
                    W = 512 * len(kbs)
                    pt = ptp.tile([128, 1024], B16, tag="pt", name="pt")
                    nc.scalar.activation(pt[:, w0:W], ps_s[:, w0:W], EXP, scale=0.125)
                    for i, kb in enumerate(kbs):
                        o = kb - 4 * J
                        if o > 0:
                            nc.vector.memset(pt[:, 512 * i:512 * i + 128 * o], 0.0)
                        nc.tensor.matmul(
                            ps_o[0:65, :],
                            vt[kb][:, 65 * h:65 * h + 65],
                            pt[:, 512 * i:512 * i + 512],
                            start=(kb == 0), stop=(kb == nkb - 1),
                        )

                # software-pipelined emission: S(g+1) before A.V(g)
                stage.append((groups[0], *s_group(groups[0])))
                for g in range(len(groups)):
                    if g + 1 < len(groups):
                        stage.append((groups[g + 1], *s_group(groups[g + 1])))
                    av_group(*stage[g])

                # custom-DVE ops mis-read PSUM at nonzero base partition on HW:
                # stage the denominator row through SBUF first.
                den = nrm.tile([1, 512], F32, tag="den", name="den")
                nc.vector.tensor_copy(den[:], ps_o[64:65, :])
                rden = nrm.tile([1, 512], F32, tag="rden", name="rden")
                nc.vector.reciprocal_approx_fast(out=rden[:], in_=den[:])
                rbc = nrm.tile([64, 512], F32, tag="rbc", name="rbc")
                nc.gpsimd.partition_broadcast(rbc[:], rden[:], channels=64)
                nc.vector.tensor_tensor(
                    out=aT[p][pb:pb + 64, q_sl], in0=ps_o[0:64, :], in1=rbc[:],
                    op=MULT)

            def emit_cproj():
                for qb in range(S // 128):
                    osb = nrm.tile([128, D], F32, tag="osb", name="osb")
                    for nb in range(2):
                        ps = mm_ps.tile([128, 512], F32, tag="mm", name="mm")
                        pc = ps[:, 0:384]
                        for c in range(PAIRS):
                            nc.tensor.matmul(
                                pc, aT[c][:, 128 * qb:128 * qb + 128],
                                wp[:, c, 384 * nb:384 * nb + 384],
                                start=(c == 0), stop=(c == PAIRS - 1))
                        nc.vector.tensor_copy(osb[:, 384 * nb:384 * nb + 384], pc)
                    nc.sync.dma_start(out_d[128 * qb:128 * qb + 128, :], osb[:])

            emit_qk_pair(0)
            emit_v()
            for p in range(PAIRS):
                if p > 0:
                    emit_qk_pair(p)
                for hh in range(2):
                    for J in range(NQ):
                        emit_head_J(p, hh, J)
            emit_cproj()

    nc.compile()
    return nc


def _host_inputs(x, c_attn_w, c_attn_b, c_proj_w, c_proj_b):
    """Slice/cast per-core inputs. Core c: batch c//2, heads 6*(c%2)..+6."""
    wq = c_attn_w[:, 0:D]
    wk = c_attn_w[:, D:2 * D]
    wv = c_attn_w[:, 2 * D:3 * D]
    bq = c_attn_b[0, 0:D]
    bk = c_attn_b[0, D:2 * D]
    bv = c_attn_b[0, 2 * D:3 * D]

    # S^T layout: rows = keys, cols = queries; mask keys above the diagonal
    mask = np.tril(np.full((128, 128), -1.0e5, dtype=np.float32), -1)

    per_hg = []
    for hg in range(2):
        g0 = HPC * hg
        cs = slice(DH * g0, DH * (g0 + HPC))  # 384 columns of this head group
        wqk = np.concatenate([wq[:, cs], wk[:, cs]], axis=1).astype(BF16)
        wqkb = np.stack(
            [np.concatenate([bq[cs], bk[cs]])[128 * m:128 * m + 128]
             for m in range(6)], axis=1).astype(np.float32)
        wva = np.zeros((D + 1, HPC * 65), dtype=np.float32)
        for j in range(HPC):
            wva[0:D, 65 * j:65 * j + 64] = wv[:, DH * (g0 + j):DH * (g0 + j + 1)]
            wva[D, 65 * j:65 * j + 64] = bv[DH * (g0 + j):DH * (g0 + j + 1)]
            wva[D, 65 * j + 64] = 1.0
        wp = c_proj_w[cs, :].astype(BF16)
        per_hg.append(dict(
            wqk=np.ascontiguousarray(wqk),
            wqkb=np.ascontiguousarray(wqkb),
            wva=np.ascontiguousarray(wva.astype(BF16)),
            wp=np.ascontiguousarray(wp),
            mask=mask,
        ))

    in_maps = []
    for c in range(NCORES):
        b, hg = divmod(c, 2)
        m = dict(per_hg[hg])
        m["xt"] = np.ascontiguousarray(x[b].T.astype(BF16))
        in_maps.append(m)
    return in_maps


def _get_executor():
    """Build the program once and cache a jitted 8-core executor.

    Mirrors bass2jax.run_bass_via_pjrt's multi-core branch, but keeps the
    jitted function alive so repeat calls reuse the compiled executable.
    """
    if "exec" in _COMPILED:
        return _COMPILED["exec"]

    import jax
    import jax.numpy as jnp  # noqa: F401
    from jax.sharding import Mesh, PartitionSpec
    from jax.experimental.shard_map import shard_map
    import concourse.mybir as mybir
    from concourse import bass2jax

    nc = _build_program()
    bass2jax.install_neuronx_cc_hook()

    part_name = nc.partition_id_tensor.name if nc.partition_id_tensor else None
    in_names, out_names, out_avals, zero_outs = [], [], [], []
    for alloc in nc.m.functions[0].allocations:
        if not isinstance(alloc, mybir.MemoryLocationSet):
            continue
        name = alloc.memorylocations[0].name
        if alloc.kind == "ExternalInput":
            if name != part_name:
                in_names.append(name)
        elif alloc.kind == "ExternalOutput":
            out_names.append(name)
            shape = tuple(alloc.tensor_shape)
            dtype = mybir.dt.np(alloc.dtype)
            out_avals.append(jax.core.ShapedArray(shape, dtype))
            zero_outs.append(np.zeros(shape, dtype))
    n_params = len(in_names)
    n_outs = len(out_avals)
    all_names = in_names + out_names
    if part_name is not None:
        all_names = all_names + [part_name]
    donate = tuple(range(n_params, n_params + n_outs))

    def _body(*args):
        operands = list(args)
        if part_name is not None:
            operands.append(bass2jax.partition_id_tensor())
        outs = bass2jax._bass_exec_p.bind(
            *operands,
            out_avals=tuple(out_avals),
            in_names=tuple(all_names),
            out_names=tuple(out_names),
            lowering_input_output_aliases=(),
            sim_require_finite=True,
            sim_require_nnan=True,
            nc=nc,
        )
        return tuple(outs)

    devices = jax.devices()[:NCORES]
    mesh = Mesh(np.asarray(devices), ("core",))
    sharded = jax.jit(
        shard_map(
            _body, mesh=mesh,
            in_specs=(PartitionSpec("core"),) * (n_params + n_outs),
            out_specs=(PartitionSpec("core"),) * n_outs,
            check_rep=False,
        ),
        donate_argnums=donate, keep_unused=True,
    )

    def run(in_maps, device_out=False):
        concat_in = [
            np.concatenate([np.asarray(in_maps[c][nm]) for c in range(NCORES)],
                           axis=0)
            for nm in in_names
        ]
        concat_zeros = [
            np.zeros((NCORES * z.shape[0], *z.shape[1:]), z.dtype)
            for z in zero_outs
        ]
        out_arrs = sharded(*concat_in, *concat_zeros)
        if device_out:
            return out_arrs
        return [
            {nm: np.asarray(out_arrs[i]).reshape(NCORES, *out_avals[i].shape)[c]
             for i, nm in enumerate(out_names)}
            for c in range(NCORES)
        ]

    _COMPILED["exec"] = run
    return run


def kernel(x, c_attn_w, c_attn_b, c_proj_w, c_proj_b):
    run = _get_executor()
    in_maps = _host_inputs(
        np.asarray(x), np.asarray(c_attn_w), np.asarray(c_attn_b),
        np.asarray(c_proj_w), np.asarray(c_proj_b))
    results = run(in_maps)

    out = np.empty((B, S, D), dtype=np.float32)
    bias = np.asarray(c_proj_b, dtype=np.float32).reshape(1, D)
    for b in range(B):
        out[b] = results[2 * b]["out"] + results[2 * b + 1]["out"] + bias
    return out


# revision 9
# speedup vs baseline: 16.0958x; 16.0958x over previous
"""GPT-2 style causal self-attention block on 8 Trainium2 NeuronCores.

Problem: x[4,2048,768] -> qkv = x@c_attn_w + b -> 12-head causal attention
-> a@c_proj_w + b.  Sharding: batch (4) x head-group (2x6 heads) = 8 cores.
Each core computes its batch's qkv columns for its 6 heads, runs attention
for those heads over the full sequence, and produces a partial c_proj
output (contraction over its 384 of 768 a-dims).  The two partials per
batch are summed on the host (+ c_proj bias).

Per-core layouts (all matmul operands bf16, f32 PSUM accumulation):
  qT,kT  [384, 2048]  head-dim on partitions (3 chunks of 128 = head pairs)
  V_aug  [2048, 390]  rows on partitions, per head 64 v-cols + ones col
                      (ones col + biases baked in via an appended ones row
                      of xT and a bias row in the weight)
  S^T    [128k, 512q] psum blocks; exp on ACT (scale=1/8 folded in);
                      causal diag masked via DVE add of a [128,128] tri mask
  A.V    psum [65, 512] accumulated over k-blocks; row 64 = softmax denom
  out^T  bf16 = num * reciprocal(denom) broadcast; feeds c_proj as lhsT
"""

import numpy as np
import ml_dtypes

B, S, D = 4, 2048, 768
NH, DH = 12, 64
NCORES = 8
HPC = 6          # heads per core
PAIRS = 3        # head pairs per core
NQ = S // 512    # q superblocks
NKB = S // 128   # k blocks
BF16 = ml_dtypes.bfloat16

_COMPILED = {}


def _build_program():
    import concourse.mybir as mybir
    import concourse.tile as tile
    from concourse import bacc

    F32, B16 = mybir.dt.float32, mybir.dt.bfloat16
    EXP = mybir.ActivationFunctionType.Exp
    ADD, MULT = mybir.AluOpType.add, mybir.AluOpType.mult

    nc = bacc.Bacc(None, target_bir_lowering=False, debug=False)
    xt_d = nc.dram_tensor("xt", [D, S], B16, kind="ExternalInput")
    wqk_d = nc.dram_tensor("wqk", [D, 768], B16, kind="ExternalInput")
    wqkb_d = nc.dram_tensor("wqkb", [128, 6], F32, kind="ExternalInput")
    wva_d = nc.dram_tensor("wva", [D + 1, HPC * 65], B16, kind="ExternalInput")
    wp_d = nc.dram_tensor("wp", [PAIRS * 128, D], B16, kind="ExternalInput")
    mask_d = nc.dram_tensor("mask", [128, 128], F32, kind="ExternalInput")
    out_d = nc.dram_tensor("out", [S, D], F32, kind="ExternalOutput")

    KC = D // 128  # 6 contraction chunks

    with tile.TileContext(nc) as tc:
        with (
            tc.tile_pool(name="const", bufs=1) as cst,
            tc.tile_pool(name="acts", bufs=1) as acts,
            tc.tile_pool(name="pt", bufs=4) as ptp,
            tc.tile_pool(name="nrm", bufs=3) as nrm,
            tc.tile_pool(name="mm_ps", bufs=2, space="PSUM") as mm_ps,
            tc.tile_pool(name="s_ps", bufs=2, space="PSUM") as s_ps,
            tc.tile_pool(name="o_ps", bufs=2, space="PSUM") as o_ps,
        ):
            xt = cst.tile([128, KC, S], B16, tag="xt", name="xt")
            ones = cst.tile([1, S], B16, tag="ones", name="ones")
            wqk = cst.tile([128, KC, 768], B16, tag="wqk", name="wqk")
            wqkb = cst.tile([128, 6], F32, tag="wqkb", name="wqkb")
            wva = cst.tile([128, KC, HPC * 65], B16, tag="wva", name="wva")
            wvab = cst.tile([1, HPC * 65], B16, tag="wvab", name="wvab")
            wp = cst.tile([128, PAIRS, D], B16, tag="wp", name="wp")
            mask = cst.tile([128, 128], F32, tag="mask", name="mask")

            nc.sync.dma_start(xt[:], xt_d.rearrange("(c p) n -> p c n", p=128))
            nc.sync.dma_start(wqk[:], wqk_d.rearrange("(c p) n -> p c n", p=128))
            nc.sync.dma_start(wqkb[:], wqkb_d[:])
            nc.sync.dma_start(wva[:], wva_d[0:D].rearrange("(c p) n -> p c n", p=128))
            nc.sync.dma_start(wvab[:], wva_d[D:D + 1])
            nc.sync.dma_start(wp[:], wp_d.rearrange("(c p) n -> p c n", p=128))
            nc.sync.dma_start(mask[:], mask_d[:])
            nc.vector.memset(ones[:], 1.0)

            qT = [acts.tile([128, S], B16, tag=f"qT{p}", name=f"qT{p}") for p in range(PAIRS)]
            kT = [acts.tile([128, S], B16, tag=f"kT{p}", name=f"kT{p}") for p in range(PAIRS)]
            vt = [acts.tile([128, HPC * 65], B16, tag=f"v{r}", name=f"v{r}") for r in range(NKB)]
            aT = [acts.tile([128, S], B16, tag=f"aT{p}", name=f"aT{p}") for p in range(PAIRS)]

            def emit_qk_pair(p):
                for dst, m in ((qT[p], p), (kT[p], PAIRS + p)):
                    for n in range(NQ):
                        ps = mm_ps.tile([128, 512], F32, tag="mm", name="mm")
                        for c in range(KC):
                            nc.tensor.matmul(
                                ps[:],
                                wqk[:, c, 128 * m:128 * m + 128],
                                xt[:, c, 512 * n:512 * n + 512],
                                start=(c == 0), stop=(c == KC - 1),
                            )
                        nc.vector.tensor_scalar_add(
                            dst[:, 512 * n:512 * n + 512], ps[:], wqkb[:, m:m + 1])

            def emit_v():
                for r in range(NKB):
                    ps = mm_ps.tile([128, 512], F32, tag="mm", name="mm")
                    pv = ps[:, 0:HPC * 65]
                    for c in range(KC):
                        nc.tensor.matmul(
                            pv, xt[:, c, 128 * r:128 * r + 128], wva[:, c, :],
                            start=(c == 0), stop=False)
                    nc.tensor.matmul(
                        pv, ones[:, 128 * r:128 * r + 128], wvab[:],
                        start=False, stop=True)
                    nc.vector.tensor_copy(vt[r][:], pv)

            def emit_head_J(p, hh, J):
                """One (head, q-superblock): S^T blocks, exp, A.V, normalize."""
                h = 2 * p + hh
                pb = 64 * hh  # partition base of this head in its pair chunk
                nkb = 4 * J + 4
                q_sl = slice(512 * J, 512 * J + 512)
                ps_o = o_ps.tile([128, 512], F32, tag="o", name="o")

                groups = [list(range(g, min(g + 2, nkb))) for g in range(0, nkb, 2)]
                stage = []  # (kbs, ps_s, window_start)

                def s_group(kbs):
                    ps_s = s_ps.tile([128, 1024], F32, tag="s", name="s")
                    w0 = 1024
                    for i, kb in enumerate(kbs):
                        sl = slice(512 * i, 512 * i + 512)
                        nc.tensor.matmul(
                            ps_s[:, sl],
                            kT[p][pb:pb + 64, 128 * kb:128 * kb + 128],
                            qT[p][pb:pb + 64, q_sl],
                            start=True, stop=True,
                            tile_position=(pb, 0),
                        )
                        o = kb - 4 * J
                        if o >= 0:  # diagonal block: add triangular mask
                            nc.vector.tensor_tensor(
                                out=ps_s[:, 512 * i + 128 * o:512 * i + 128 * o + 128],
                                in0=ps_s[:, 512 * i + 128 * o:512 * i + 128 * o + 128],
                                in1=mask[:], op=ADD)
                        # window start: skip 512-col slices that are fully masked
                        w0 = min(w0, 512 * i + (128 * o if o > 0 else 0))
                    return ps_s, w0

                def av_group(kbs, ps_s, w0):
                    W = 512 * len(kbs)
                    pt = ptp.tile([128, 1024], B16, tag="pt", name="pt")
                    nc.scalar.activation(pt[:, w0:W], ps_s[:, w0:W], EXP, scale=0.125)
                    for i, kb in enumerate(kbs):
                        o = kb - 4 * J
                        if o > 0:
                            nc.vector.memset(pt[:, 512 * i:512 * i + 128 * o], 0.0)
                        nc.tensor.matmul(
                            ps_o[0:65, :],
                            vt[kb][:, 65 * h:65 * h + 65],
                            pt[:, 512 * i:512 * i + 512],
                            start=(kb == 0), stop=(kb == nkb - 1),
                        )

                # software-pipelined emission: S(g+1) before A.V(g)
                stage.append((groups[0], *s_group(groups[0])))
                for g in range(len(groups)):
                    if g + 1 < len(groups):
                        stage.append((groups[g + 1], *s_group(groups[g + 1])))
                    av_group(*stage[g])

                # custom-DVE ops mis-read PSUM at nonzero base partition on HW:
                # stage the denominator row through SBUF first.
                den = nrm.tile([1, 512], F32, tag="den", name="den")
                nc.vector.tensor_copy(den[:], ps_o[64:65, :])
                rden = nrm.tile([1, 512], F32, tag="rden", name="rden")
                nc.vector.reciprocal_approx_fast(out=rden[:], in_=den[:])
                rbc = nrm.tile([64, 512], F32, tag="rbc", name="rbc")
                nc.gpsimd.partition_broadcast(rbc[:], rden[:], channels=64)
                nc.vector.tensor_tensor(
                    out=aT[p][pb:pb + 64, q_sl], in0=ps_o[0:64, :], in1=rbc[:],
                    op=MULT)

            def emit_cproj():
                for qb in range(S // 128):
                    osb = nrm.tile([128, D], F32, tag="osb", name="osb")
                    for nb in range(2):
                        ps = mm_ps.tile([128, 512], F32, tag="mm", name="mm")
                        pc = ps[:, 0:384]
                        for c in range(PAIRS):
                            nc.tensor.matmul(
                                pc, aT[c][:, 128 * qb:128 * qb + 128],
                                wp[:, c, 384 * nb:384 * nb + 384],
                                start=(c == 0), stop=(c == PAIRS - 1))
                        nc.vector.tensor_copy(osb[:, 384 * nb:384 * nb + 384], pc)
                    nc.sync.dma_start(out_d[128 * qb:128 * qb + 128, :], osb[:])

            emit_qk_pair(0)
            emit_v()
            for p in range(PAIRS):
                if p > 0:
                    emit_qk_pair(p)
                for hh in range(2):
                    for J in range(NQ):
                        emit_head_J(p, hh, J)
            emit_cproj()

    nc.compile()
    return nc


def _host_inputs(x, c_attn_w, c_attn_b, c_proj_w, c_proj_b):
    """Slice/cast per-core inputs. Core c: batch c//2, heads 6*(c%2)..+6."""
    wq = c_attn_w[:, 0:D]
    wk = c_attn_w[:, D:2 * D]
    wv = c_attn_w[:, 2 * D:3 * D]
    bq = c_attn_b[0, 0:D]
    bk = c_attn_b[0, D:2 * D]
    bv = c_attn_b[0, 2 * D:3 * D]

    # S^T layout: rows = keys, cols = queries; mask keys above the diagonal
    mask = np.tril(np.full((128, 128), -1.0e5, dtype=np.float32), -1)

    per_hg = []
    for hg in range(2):
        g0 = HPC * hg
        cs = slice(DH * g0, DH * (g0 + HPC))  # 384 columns of this head group
        wqk = np.concatenate([wq[:, cs], wk[:, cs]], axis=1).astype(BF16)
        wqkb = np.stack(
            [np.concatenate([bq[cs], bk[cs]])[128 * m:128 * m + 128]
             for m in range(6)], axis=1).astype(np.float32)
        wva = np.zeros((D + 1, HPC * 65), dtype=np.float32)
        for j in range(HPC):
            wva[0:D, 65 * j:65 * j + 64] = wv[:, DH * (g0 + j):DH * (g0 + j + 1)]
            wva[D, 65 * j:65 * j + 64] = bv[DH * (g0 + j):DH * (g0 + j + 1)]
            wva[D, 65 * j + 64] = 1.0
        wp = c_proj_w[cs, :].astype(BF16)
        per_hg.append(dict(
            wqk=np.ascontiguousarray(wqk),
            wqkb=np.ascontiguousarray(wqkb),
            wva=np.ascontiguousarray(wva.astype(BF16)),
            wp=np.ascontiguousarray(wp),
            mask=mask,
        ))

    in_maps = []
    for c in range(NCORES):
        b, hg = divmod(c, 2)
        m = dict(per_hg[hg])
        m["xt"] = np.ascontiguousarray(x[b].T.astype(BF16))
        in_maps.append(m)
    return in_maps


def _get_executor():
    """Build the program once and cache a jitted 8-core executor.

    Mirrors bass2jax.run_bass_via_pjrt's multi-core branch, but keeps the
    jitted function alive so repeat calls reuse the compiled executable.
    """
    if "exec" in _COMPILED:
        return _COMPILED["exec"]

    import jax
    import jax.numpy as jnp  # noqa: F401
    from jax.sharding import Mesh, PartitionSpec
    from jax.experimental.shard_map import shard_map
    import concourse.mybir as mybir
    from concourse import bass2jax

    nc = _build_program()
    bass2jax.install_neuronx_cc_hook()

    part_name = nc.partition_id_tensor.name if nc.partition_id_tensor else None
    in_names, out_names, out_avals, zero_outs = [], [], [], []
    for alloc in nc.m.functions[0].allocations:
        if not isinstance(alloc, mybir.MemoryLocationSet):
            continue
        name = alloc.memorylocations[0].name
        if alloc.kind == "ExternalInput":
            if name != part_name:
                in_names.append(name)
        elif alloc.kind == "ExternalOutput":
            out_names.append(name)
            shape = tuple(alloc.tensor_shape)
            dtype = mybir.dt.np(alloc.dtype)
            out_avals.append(jax.core.ShapedArray(shape, dtype))
            zero_outs.append(np.zeros(shape, dtype))
    n_params = len(in_names)
    n_outs = len(out_avals)
    all_names = in_names + out_names
    if part_name is not None:
        all_names = all_names + [part_name]
    donate = tuple(range(n_params, n_params + n_outs))

    def _body(*args):
        operands = list(args)
        if part_name is not None:
            operands.append(bass2jax.partition_id_tensor())
        outs = bass2jax._bass_exec_p.bind(
            *operands,
            out_avals=tuple(out_avals),
            in_names=tuple(all_names),
            out_names=tuple(out_names),
            lowering_input_output_aliases=(),
            sim_require_finite=True,
            sim_require_nnan=True,
            nc=nc,
        )
        return tuple(outs)

    devices = jax.devices()[:NCORES]
    mesh = Mesh(np.asarray(devices), ("core",))
    sharded = jax.jit(
        shard_map(
            _body, mesh=mesh,
            in_specs=(PartitionSpec("core"),) * (n_params + n_outs),
            out_specs=(PartitionSpec("core"),) * n_outs,
            check_rep=False,
        ),
        donate_argnums=donate, keep_unused=True,
    )

    def run(in_maps, device_out=False):
        concat_in = [
            np.concatenate([np.asarray(in_maps[c][nm]) for c in range(NCORES)],
                           axis=0)
            for nm in in_names
        ]
        concat_zeros = [
            np.zeros((NCORES * z.shape[0], *z.shape[1:]), z.dtype)
            for z in zero_outs
        ]
        out_arrs = sharded(*concat_in, *concat_zeros)
        if device_out:
            return out_arrs
        return [
            {nm: np.asarray(out_arrs[i]).reshape(NCORES, *out_avals[i].shape)[c]
             for i, nm in enumerate(out_names)}
            for c in range(NCORES)
        ]

    run.sharded = sharded
    run.in_names = in_names
    run.out_avals = out_avals
    run.zero_shapes = [
        ((NCORES * z.shape[0], *z.shape[1:]), z.dtype) for z in zero_outs
    ]
    _COMPILED["exec"] = run
    return run


def kernel(x, c_attn_w, c_attn_b, c_proj_w, c_proj_b):
    run = _get_executor()
    in_maps = _host_inputs(
        np.asarray(x), np.asarray(c_attn_w), np.asarray(c_attn_b),
        np.asarray(c_proj_w), np.asarray(c_proj_b))
    results = run(in_maps)

    out = np.empty((B, S, D), dtype=np.float32)
    bias = np.asarray(c_proj_b, dtype=np.float32).reshape(1, D)
    for b in range(B):
        out[b] = results[2 * b]["out"] + results[2 * b + 1]["out"] + bias
    return out


# revision 31
# speedup vs baseline: 6954.8759x; 432.0938x over previous
"""GPT-2 style causal self-attention block on 8 Trainium2 NeuronCores.

Problem: x[4,2048,768] -> qkv = x@c_attn_w + b -> 12-head causal attention
-> a@c_proj_w + b.  Sharding: batch (4) x head-group (2x6 heads) = 8 cores.
Each core computes its batch's qkv columns for its 6 heads, runs attention
for those heads over the full sequence, and produces a partial c_proj
output (contraction over its 384 of 768 a-dims).  The two partials per
batch are summed on the host (+ c_proj bias).

Per-core layouts (all matmul operands bf16, f32 PSUM accumulation):
  qT,kT  [384, 2048]  head-dim on partitions (3 chunks of 128 = head pairs;
                      the two heads of a chunk run as concurrent K=64
                      row-packed matmuls via tile_position 0/64)
  V_aug  [2048, 390]  rows on partitions, per head 64 v-cols + ones col
                      (ones col + biases baked in via an appended ones row
                      of xT and a bias row in the weight)
  S^T    [128k, 512q] psum blocks, fully-masked left columns skipped; exp on
                      ACT (scale=1/8 folded in); causal diagonal fixed
                      post-exp by a 0/1 tri-mask multiply (DVE bf16 2x)
  A.V    psum [65, 512] accumulated over k-blocks; row 64 = softmax denom
  out^T  bf16 = num * reciprocal(denom) broadcast; feeds c_proj as lhsT

Measured: ~210 us/core HW (slope of on-chip rep-loop), vs ~107 us pure
matmul roofline; ACT exp throughput (~120 us) and the S->exp->A.V chain
are the co-bottlenecks with PE (~145 us real).
"""

import numpy as np
import ml_dtypes

B, S, D = 4, 2048, 768
NH, DH = 12, 64
NCORES = 8
HPC = 6          # heads per core
PAIRS = 3        # head pairs per core
NQ = S // 512    # q superblocks
NKB = S // 128   # k blocks
BF16 = ml_dtypes.bfloat16

_COMPILED = {}


def _build_program(reps=1, spread_qk=True):
    import contextlib
    import concourse.mybir as mybir
    import concourse.tile as tile
    from concourse import bacc

    F32, B16 = mybir.dt.float32, mybir.dt.bfloat16
    EXP = mybir.ActivationFunctionType.Exp
    ADD, MULT = mybir.AluOpType.add, mybir.AluOpType.mult

    nc = bacc.Bacc(None, target_bir_lowering=False, debug=False)
    xt_d = nc.dram_tensor("xt", [D, S], B16, kind="ExternalInput")
    wqk_d = nc.dram_tensor("wqk", [D, 768], B16, kind="ExternalInput")
    wqkb_d = nc.dram_tensor("wqkb", [128, 6], F32, kind="ExternalInput")
    wva_d = nc.dram_tensor("wva", [D + 1, HPC * 65], B16, kind="ExternalInput")
    wp_d = nc.dram_tensor("wp", [PAIRS * 128, D], B16, kind="ExternalInput")
    mask_d = nc.dram_tensor("mask", [128, 128], B16, kind="ExternalInput")
    out_d = nc.dram_tensor("out", [S, D], F32, kind="ExternalOutput")

    KC = D // 128  # 6 contraction chunks

    with tile.TileContext(nc) as tc:
        with (
            tc.tile_pool(name="const", bufs=1) as cst,
            tc.tile_pool(name="acts", bufs=1) as acts,
            tc.tile_pool(name="pt", bufs=6) as ptp,
            tc.tile_pool(name="nrm", bufs=3) as nrm,
            tc.tile_pool(name="mm_ps", bufs=2, space="PSUM") as mm_ps,
            tc.tile_pool(name="s_ps", bufs=2, space="PSUM") as s_ps,
            tc.tile_pool(name="o_ps", bufs=2, space="PSUM") as o_ps,
        ):
            xt = cst.tile([128, KC, S], B16, tag="xt", name="xt")
            ones = cst.tile([1, S], B16, tag="ones", name="ones")
            wqk = cst.tile([128, KC, 768], B16, tag="wqk", name="wqk")
            wqkb = cst.tile([128, 6], F32, tag="wqkb", name="wqkb")
            wva = cst.tile([128, KC, HPC * 65], B16, tag="wva", name="wva")
            wvab = cst.tile([1, HPC * 65], B16, tag="wvab", name="wvab")
            wp = cst.tile([128, PAIRS, D], B16, tag="wp", name="wp")
            mask = cst.tile([128, 128], B16, tag="mask", name="mask")

            # piecewise DMAs ordered by first use so matmuls start early
            for c in range(KC):
                nc.sync.dma_start(wqk[:, c, :], wqk_d[128 * c:128 * c + 128, :])
            for n in range(NQ):
                for c in range(KC):
                    nc.sync.dma_start(
                        xt[:, c, 512 * n:512 * n + 512],
                        xt_d[128 * c:128 * c + 128, 512 * n:512 * n + 512])
            nc.sync.dma_start(wqkb[:], wqkb_d[:])
            nc.sync.dma_start(mask[:], mask_d[:])
            for c in range(KC):
                nc.sync.dma_start(wva[:, c, :], wva_d[128 * c:128 * c + 128, :])
            nc.sync.dma_start(wvab[:], wva_d[D:D + 1])
            for c in range(PAIRS):
                nc.sync.dma_start(wp[:, c, :], wp_d[128 * c:128 * c + 128, :])
            nc.vector.memset(ones[:], 1.0)

            qT = [[acts.tile([128, 512], B16, tag=f"qT{p}_{n}", name=f"qT{p}_{n}")
                   for n in range(NQ)] for p in range(PAIRS)]
            kT = [[acts.tile([128, 512], B16, tag=f"kT{p}_{n}", name=f"kT{p}_{n}")
                   for n in range(NQ)] for p in range(PAIRS)]
            vt = [acts.tile([128, HPC * 65], B16, tag=f"v{r}", name=f"v{r}") for r in range(NKB)]
            aT = [[acts.tile([128, 512], B16, tag=f"aT{p}_{n}", name=f"aT{p}_{n}")
                   for n in range(NQ)] for p in range(PAIRS)]

            def emit_qk_n(p, n):
                for dst, m in ((qT[p][n], p), (kT[p][n], PAIRS + p)):
                    ps = mm_ps.tile([128, 512], F32, tag="mm", name="mm")
                    for c in range(KC):
                        nc.tensor.matmul(
                            ps[:],
                            wqk[:, c, 128 * m:128 * m + 128],
                            xt[:, c, 512 * n:512 * n + 512],
                            start=(c == 0), stop=(c == KC - 1),
                        )
                    nc.vector.tensor_scalar_add(dst[:], ps[:], wqkb[:, m:m + 1])

            def emit_qk_pair(p):
                for n in range(NQ):
                    emit_qk_n(p, n)

            def emit_v(rows):
                for r in rows:
                    ps = mm_ps.tile([128, 512], F32, tag="mm", name="mm")
                    pv = ps[:, 0:HPC * 65]
                    for c in range(KC):
                        nc.tensor.matmul(
                            pv, xt[:, c, 128 * r:128 * r + 128], wva[:, c, :],
                            start=(c == 0), stop=False)
                    nc.tensor.matmul(
                        pv, ones[:, 128 * r:128 * r + 128], wvab[:],
                        start=False, stop=True)
                    nc.vector.tensor_copy(vt[r][:], pv)

            def emit_head_J(p, hh, J):
                """One (head, q-superblock): S^T blocks, exp, A.V, normalize."""
                h = 2 * p + hh
                pb = 64 * hh  # partition base of this head in its pair chunk
                nkb = 4 * J + 4
                ps_o = o_ps.tile([128, 512], F32, tag="o", name="o")

                groups = [list(range(g, min(g + 2, nkb))) for g in range(0, nkb, 2)]
                stage = []  # (kbs, ps_s, window_start)

                def s_group(kbs):
                    ps_s = s_ps.tile([128, 1024], F32, tag="s", name="s")
                    for i, kb in enumerate(kbs):
                        o = max(kb - 4 * J, 0)  # skip fully-masked left columns
                        nc.tensor.matmul(
                            ps_s[:, 512 * i + 128 * o:512 * i + 512],
                            kT[p][kb // 4][pb:pb + 64,
                                           128 * (kb % 4):128 * (kb % 4) + 128],
                            qT[p][J][pb:pb + 64, 128 * o:],
                            start=True, stop=True,
                            tile_position=(pb, 0),
                        )
                    return ps_s, 0

                def av_group(kbs, ps_s, w0):
                    pt = ptp.tile([128, 1024], B16, tag="pt", name="pt")
                    # exp: one call over contiguous valid region when no gaps,
                    # else exact per-kb windows (diagonal groups)
                    offs = [max(kb - 4 * J, 0) * 128 for kb in kbs]
                    if all(o == 0 for o in offs):
                        nc.scalar.activation(pt[:, 0:512 * len(kbs)],
                                             ps_s[:, 0:512 * len(kbs)],
                                             EXP, scale=0.125)
                    else:
                        for i, o in enumerate(offs):
                            nc.scalar.activation(
                                pt[:, 512 * i + o:512 * i + 512],
                                ps_s[:, 512 * i + o:512 * i + 512],
                                EXP, scale=0.125)
                    for i, kb in enumerate(kbs):
                        o = kb - 4 * J
                        if o >= 0:  # causal 0/1 mask applied post-exp (bf16 2x)
                            d_sl = slice(512 * i + 128 * o, 512 * i + 128 * o + 128)
                            nc.vector.tensor_tensor(
                                out=pt[:, d_sl], in0=pt[:, d_sl], in1=mask[:],
                                op=MULT)
                        if o > 0:
                            nc.gpsimd.memset(pt[:, 512 * i:512 * i + 128 * o], 0.0)
                        nc.tensor.matmul(
                            ps_o[0:65, :],
                            vt[kb][:, 65 * h:65 * h + 65],
                            pt[:, 512 * i:512 * i + 512],
                            start=(kb == 0), stop=(kb == nkb - 1),
                        )

                # software-pipelined emission: S(g+1) before A.V(g)
                stage.append((groups[0], *s_group(groups[0])))
                for g in range(len(groups)):
                    if g + 1 < len(groups):
                        stage.append((groups[g + 1], *s_group(groups[g + 1])))
                    av_group(*stage[g])

                # custom-DVE ops mis-read PSUM at nonzero base partition on HW:
                # stage the denominator row through SBUF first.
                den = nrm.tile([1, 512], F32, tag="den", name="den")
                nc.vector.tensor_copy(den[:], ps_o[64:65, :])
                rden = nrm.tile([1, 512], F32, tag="rden", name="rden")
                nc.vector.reciprocal_approx_fast(out=rden[:], in_=den[:])
                rbc = nrm.tile([64, 512], F32, tag="rbc", name="rbc")
                nc.gpsimd.partition_broadcast(rbc[:], rden[:], channels=64)
                nc.vector.tensor_tensor(
                    out=aT[p][J][pb:pb + 64, :], in0=ps_o[0:64, :], in1=rbc[:],
                    op=MULT)

            def emit_cproj(qbs):
                for qb in qbs:
                    osb = nrm.tile([128, D], F32, tag="osb", name="osb")
                    for nb in range(2):
                        ps = mm_ps.tile([128, 512], F32, tag="mm", name="mm")
                        pc = ps[:, 0:384]
                        for c in range(PAIRS):
                            nc.tensor.matmul(
                                pc,
                                aT[c][qb // 4][:, 128 * (qb % 4):128 * (qb % 4) + 128],
                                wp[:, c, 384 * nb:384 * nb + 384],
                                start=(c == 0), stop=(c == PAIRS - 1))
                        nc.any.tensor_copy(osb[:, 384 * nb:384 * nb + 384], pc)
                    nc.sync.dma_start(out_d[128 * qb:128 * qb + 128, :], osb[:])

            loop = tc.For_i(0, reps, 1) if reps > 1 else contextlib.nullcontext()
            with loop:
                for n in range(NQ):
                    emit_qk_n(0, n)
                    emit_v(range(4 * n, 4 * n + 4))
                    emit_head_J(0, 0, n)
                    emit_head_J(0, 1, n)
                    if spread_qk:
                        emit_qk_n(1, n)  # fill PE during p0's ACT-bound rounds
                if not spread_qk:
                    emit_qk_pair(1)
                for J in range(NQ):
                    emit_head_J(1, 0, J)
                    emit_head_J(1, 1, J)
                    if spread_qk:
                        emit_qk_n(2, J)
                if not spread_qk:
                    emit_qk_pair(2)
                for J in range(NQ):
                    emit_head_J(2, 0, J)
                    emit_head_J(2, 1, J)
                    emit_cproj(range(4 * J, 4 * J + 4))

    nc.compile()
    return nc


def _host_inputs(x, c_attn_w, c_attn_b, c_proj_w, c_proj_b):
    """Slice/cast per-core inputs. Core c: batch c//2, heads 6*(c%2)..+6."""
    wq = c_attn_w[:, 0:D]
    wk = c_attn_w[:, D:2 * D]
    wv = c_attn_w[:, 2 * D:3 * D]
    bq = c_attn_b[0, 0:D]
    bk = c_attn_b[0, D:2 * D]
    bv = c_attn_b[0, 2 * D:3 * D]

    # S^T layout: rows = keys, cols = queries; keep keys <= query (0/1,
    # multiplied into exp(S^T) post-activation)
    mask = np.triu(np.ones((128, 128), dtype=np.float32)).astype(BF16)

    per_hg = []
    for hg in range(2):
        g0 = HPC * hg
        cs = slice(DH * g0, DH * (g0 + HPC))  # 384 columns of this head group
        wqk = np.concatenate([wq[:, cs], wk[:, cs]], axis=1).astype(BF16)
        wqkb = np.stack(
            [np.concatenate([bq[cs], bk[cs]])[128 * m:128 * m + 128]
             for m in range(6)], axis=1).astype(np.float32)
        wva = np.zeros((D + 1, HPC * 65), dtype=np.float32)
        for j in range(HPC):
            wva[0:D, 65 * j:65 * j + 64] = wv[:, DH * (g0 + j):DH * (g0 + j + 1)]
            wva[D, 65 * j:65 * j + 64] = bv[DH * (g0 + j):DH * (g0 + j + 1)]
            wva[D, 65 * j + 64] = 1.0
        wp = c_proj_w[cs, :].astype(BF16)
        per_hg.append(dict(
            wqk=np.ascontiguousarray(wqk),
            wqkb=np.ascontiguousarray(wqkb),
            wva=np.ascontiguousarray(wva.astype(BF16)),
            wp=np.ascontiguousarray(wp),
            mask=mask,
        ))

    in_maps = []
    for c in range(NCORES):
        b, hg = divmod(c, 2)
        m = dict(per_hg[hg])
        m["xt"] = np.ascontiguousarray(x[b].T.astype(BF16))
        in_maps.append(m)
    return in_maps


def _get_executor():
    """Build the program once and cache a jitted 8-core executor.

    Mirrors bass2jax.run_bass_via_pjrt's multi-core branch, but keeps the
    jitted function alive so repeat calls reuse the compiled executable.
    """
    if "exec" in _COMPILED:
        return _COMPILED["exec"]

    import jax
    import jax.numpy as jnp  # noqa: F401
    from jax.sharding import Mesh, PartitionSpec
    from jax.experimental.shard_map import shard_map
    import concourse.mybir as mybir
    from concourse import bass2jax

    nc = _build_program()
    bass2jax.install_neuronx_cc_hook()

    part_name = nc.partition_id_tensor.name if nc.partition_id_tensor else None
    in_names, out_names, out_avals, zero_outs = [], [], [], []
    for alloc in nc.m.functions[0].allocations:
        if not isinstance(alloc, mybir.MemoryLocationSet):
            continue
        name = alloc.memorylocations[0].name
        if alloc.kind == "ExternalInput":
            if name != part_name:
                in_names.append(name)
        elif alloc.kind == "ExternalOutput":
            out_names.append(name)
            shape = tuple(alloc.tensor_shape)
            dtype = mybir.dt.np(alloc.dtype)
            out_avals.append(jax.core.ShapedArray(shape, dtype))
            zero_outs.append(np.zeros(shape, dtype))
    n_params = len(in_names)
    n_outs = len(out_avals)
    all_names = in_names + out_names
    if part_name is not None:
        all_names = all_names + [part_name]
    donate = tuple(range(n_params, n_params + n_outs))

    def _body(*args):
        operands = list(args)
        if part_name is not None:
            operands.append(bass2jax.partition_id_tensor())
        outs = bass2jax._bass_exec_p.bind(
            *operands,
            out_avals=tuple(out_avals),
            in_names=tuple(all_names),
            out_names=tuple(out_names),
            lowering_input_output_aliases=(),
            sim_require_finite=True,
            sim_require_nnan=True,
            nc=nc,
        )
        return tuple(outs)

    devices = jax.devices()[:NCORES]
    mesh = Mesh(np.asarray(devices), ("core",))
    sharded = jax.jit(
        shard_map(
            _body, mesh=mesh,
            in_specs=(PartitionSpec("core"),) * (n_params + n_outs),
            out_specs=(PartitionSpec("core"),) * n_outs,
            check_rep=False,
        ),
        donate_argnums=donate, keep_unused=True,
    )

    def run(in_maps, device_out=False):
        concat_in = [
            np.concatenate([np.asarray(in_maps[c][nm]) for c in range(NCORES)],
                           axis=0)
            for nm in in_names
        ]
        concat_zeros = [
            np.zeros((NCORES * z.shape[0], *z.shape[1:]), z.dtype)
            for z in zero_outs
        ]
        out_arrs = sharded(*concat_in, *concat_zeros)
        if device_out:
            return out_arrs
        return [
            {nm: np.asarray(out_arrs[i]).reshape(NCORES, *out_avals[i].shape)[c]
             for i, nm in enumerate(out_names)}
            for c in range(NCORES)
        ]

    run.sharded = sharded
    run.in_names = in_names
    run.out_avals = out_avals
    run.zero_shapes = [
        ((NCORES * z.shape[0], *z.shape[1:]), z.dtype) for z in zero_outs
    ]
    _COMPILED["exec"] = run
    return run


def kernel(x, c_attn_w, c_attn_b, c_proj_w, c_proj_b):
    run = _get_executor()
    in_maps = _host_inputs(
        np.asarray(x), np.asarray(c_attn_w), np.asarray(c_attn_b),
        np.asarray(c_proj_w), np.asarray(c_proj_b))
    results = run(in_maps)

    out = np.empty((B, S, D), dtype=np.float32)
    bias = np.asarray(c_proj_b, dtype=np.float32).reshape(1, D)
    for b in range(B):
        out[b] = results[2 * b]["out"] + results[2 * b + 1]["out"] + bias
    return out
